# revision 10
# baseline (speedup 1.0000x reference)
"""Trainium2 Bass kernel for a dense pre-LN transformer block.

Block: y = x + proj(causal_mha(LN1(x))) ; out = y + FFN(LN2(y))
Shapes (hardcoded): x [4, 2048, 1024], H=16 heads, HD=64, FF=2048, fp32 I/O.

Sharding (8 cores, no collectives): core c handles batch b=c//2 and a
balanced half of the queries (role r=c%2; A: rows [0,512)+[1536,2048),
B: rows [512,1536)).  The key/value sequence is permuted on the host per
core (own rows first) so one SPMD program serves both roles; causality is
enforced by compile-time triangular affine_select masks on the diagonal
chunks plus a per-core exp-bias table (-100 => exp ~ 0) for the chunks
whose validity depends on the role.

Matmuls run in bf16 (fp32 PSUM accumulate); layernorm stats, softmax and
residuals stay fp32.  Scores are computed transposed (st[t,q]) so softmax
needs no transposes; V carries an extra ones-column so the softmax
denominator drops out of the z-matmul for free.
"""

import numpy as np
import ml_dtypes

import concourse.bass as bass
import concourse.bacc as bacc
import concourse.tile as tile
import concourse.mybir as mybir
from concourse.bass import ts
from concourse.bass_utils import run_bass_kernel_spmd
from concourse.masks import make_identity

BF16 = mybir.dt.bfloat16
F32 = mybir.dt.float32
AF = mybir.ActivationFunctionType
ALU = mybir.AluOpType

S = 2048          # sequence length
E = 1024          # embedding dim
H = 16            # heads
HD = 64           # head dim
FF = 2048         # ffn hidden
P = 128           # partitions
NQ = 1024         # queries owned per core
EPS = 1e-5
NEG = -100.0      # exp bias for masked-out chunks (exp(-100) ~ 0)

# chunk schedule (in permuted key coordinates), identical on every core:
# q-tile 0 (own positions [0,512)):   key chunks 0-3 (diag) + 8-11 (role-dep)
# q-tile 1 (own positions [512,1024)): key chunks 0-15 (4-7 diag, 12-15 role-dep)
SCHED = [[0, 1, 2, 3, 8, 9, 10, 11], list(range(16))]
DIAG = [set(range(0, 4)), set(range(4, 8))]

_CACHE = {}


def _build_program():
    nc = bacc.Bacc("TRN2", target_bir_lowering=False, debug=False)

    # ---- per-core dram inputs -------------------------------------------
    xp_d = nc.dram_tensor("xp", [S, E], F32, kind="ExternalInput")
    wq_d = nc.dram_tensor("wq2", [H // 2, E // P, P, P], BF16, kind="ExternalInput")
    wk_d = nc.dram_tensor("wk2", [H // 2, E // P, P, P], BF16, kind="ExternalInput")
    wv_d = nc.dram_tensor("wv", [E // P, P, E], BF16, kind="ExternalInput")
    wp_d = nc.dram_tensor("wp", [E // P, P, E], BF16, kind="ExternalInput")
    w1_d = nc.dram_tensor("w1", [E // P, P, FF], BF16, kind="ExternalInput")
    w2_d = nc.dram_tensor("w2", [FF // P, P, E], BF16, kind="ExternalInput")
    b1_d = nc.dram_tensor("b1t", [P, FF // P], F32, kind="ExternalInput")
    bp_d = nc.dram_tensor("bproj", [E], F32, kind="ExternalInput")
    b2_d = nc.dram_tensor("b2", [E], F32, kind="ExternalInput")
    g1_d = nc.dram_tensor("g1t", [P, E // P], F32, kind="ExternalInput")
    h1_d = nc.dram_tensor("h1t", [P, E // P], F32, kind="ExternalInput")
    g2_d = nc.dram_tensor("g2t", [P, E // P], F32, kind="ExternalInput")
    h2_d = nc.dram_tensor("h2t", [P, E // P], F32, kind="ExternalInput")
    mb_d = nc.dram_tensor("mb", [24], F32, kind="ExternalInput")
    out_d = nc.dram_tensor("out", [NQ, E], F32, kind="ExternalOutput")

    EC = E // P    # 8 e-chunks
    FC = FF // P   # 16 f-chunks
    NCH = S // P   # 16 key chunks
    HP = H // 2    # 8 head pairs

    def layernorm_to_T(tc, pools, x_ap, sc, gcol, hcol, dstT, act_pool, tp_psum):
        """LN of one [128, E] row-tile (fp32 in SBUF/psum-readable AP) then
        transpose to dstT[:, ec, sc*128:(sc+1)*128] (bf16) applying g/b in
        the transposed domain (per-partition there)."""
        nc_ = tc.nc
        small = pools["small"]
        stats = small.tile([P, 2, 6], F32, tag="bnstats")
        for g in range(2):
            nc_.vector.bn_stats(out=stats[:, g, :], in_=x_ap[:, g * 512:(g + 1) * 512])
        mv = small.tile([P, 2], F32, tag="bnaggr")
        nc_.vector.bn_aggr(out=mv, in_=stats)
        std = small.tile([P, 1], F32, tag="std")
        nc_.scalar.activation(out=std, in_=mv[:, 1:2], func=AF.Sqrt,
                              bias=pools["eps"], scale=1.0)
        rstd = small.tile([P, 1], F32, tag="rstd")
        nc_.vector.reciprocal(out=rstd, in_=std)
        nm = small.tile([P, 1], F32, tag="negmean")
        nc_.vector.scalar_tensor_tensor(out=nm, in0=mv[:, 0:1], scalar=-1.0,
                                        in1=rstd, op0=ALU.mult, op1=ALU.mult)
        tmp = act_pool.tile([P, E], BF16, tag="ln_tmp")
        nc_.scalar.activation(out=tmp, in_=x_ap, func=AF.Identity,
                              bias=nm, scale=rstd)
        for ec in range(EC):
            tp = tp_psum.tile([P, P], BF16, tag="tp")
            nc_.tensor.transpose(tp, tmp[:, ts(ec, P)], pools["ident"])
            nc_.vector.scalar_tensor_tensor(
                out=dstT[:, ec, ts(sc, P)], in0=tp, scalar=gcol[:, ec:ec + 1],
                in1=hcol[:, ec:ec + 1].to_broadcast((P, P)),
                op0=ALU.mult, op1=ALU.add)

    with tile.TileContext(nc) as tc:
        import contextlib
        stk = contextlib.ExitStack()
        with stk:
            const = stk.enter_context(tc.tile_pool(name="const", bufs=1))
            small = stk.enter_context(tc.tile_pool(name="small", bufs=4))
            dram = stk.enter_context(tc.tile_pool(name="dram", bufs=1, space="DRAM"))

            ident = const.tile([P, P], BF16)
            make_identity(nc, ident)
            eps_t = const.tile([P, 1], F32)
            nc.vector.memset(eps_t, EPS)
            mb_sb = const.tile([P, 24], F32)
            nc.gpsimd.dma_start(out=mb_sb, in_=mb_d[None, :].to_broadcast((P, 24)))
            b1_sb = const.tile([P, FC], F32)
            nc.sync.dma_start(out=b1_sb, in_=b1_d[:, :])
            bp_sb = const.tile([P, E], F32)
            nc.gpsimd.dma_start(out=bp_sb, in_=bp_d[None, :].to_broadcast((P, E)))
            b2_sb = const.tile([P, E], F32)
            nc.gpsimd.dma_start(out=b2_sb, in_=b2_d[None, :].to_broadcast((P, E)))
            g1_sb = const.tile([P, EC], F32)
            nc.sync.dma_start(out=g1_sb, in_=g1_d[:, :])
            h1_sb = const.tile([P, EC], F32)
            nc.sync.dma_start(out=h1_sb, in_=h1_d[:, :])
            g2_sb = const.tile([P, EC], F32)
            nc.sync.dma_start(out=g2_sb, in_=g2_d[:, :])
            h2_sb = const.tile([P, EC], F32)
            nc.sync.dma_start(out=h2_sb, in_=h2_d[:, :])
            pools = {"ident": ident, "eps": eps_t, "small": small}

            out1_dram = dram.tile([NQ, E], F32)
            # V streamed through DRAM: [ch, t, h, 64] values + ones col 64
            V_dram = dram.tile([NCH, P, H, HD + 1], BF16)

            # late-phase persistent buffers (allocated first = bottom of stack)
            late = stk.enter_context(tc.tile_pool(name="late", bufs=1))
            ln2T = late.tile([P, EC, NQ], BF16)
            zT = late.tile([P, EC, NQ], BF16)

            with tc.tile_pool(name="attn", bufs=1) as attn:

                KT = attn.tile([P, HP, S], BF16)
                QT = attn.tile([P, HP, NQ], BF16)
                ones16 = attn.tile([P, H], BF16)
                nc.vector.memset(ones16, 1.0)
                for ch in range(NCH):
                    nc.sync.dma_start(out=V_dram[ch, :, :, HD:HD + 1],
                                      in_=ones16[:, :, None])

                with tc.tile_pool(name="lnT_pool", bufs=1) as lnT_pool, \
                     tc.tile_pool(name="xstream", bufs=3) as xstream, \
                     tc.tile_pool(name="acts", bufs=3) as acts, \
                     tc.tile_pool(name="wstream", bufs=2) as wstream, \
                     tc.tile_pool(name="wv_pool", bufs=1) as wv_pool, \
                     tc.tile_pool(name="tp_psum", bufs=2, space="PSUM") as tp_psum, \
                     tc.tile_pool(name="mm_psum", bufs=2, space="PSUM") as mm_psum:
                    lnT = lnT_pool.tile([P, EC, S], BF16)

                    # ---- phase 1: LN1 over all rows -> lnT [e, s] -------
                    for sc in range(S // P):
                        xt = xstream.tile([P, E], F32, tag="x")
                        nc.sync.dma_start(out=xt, in_=xp_d[ts(sc, P), :])
                        layernorm_to_T(tc, pools, xt, sc, g1_sb, h1_sb, lnT,
                                       acts, tp_psum)

                    # ---- phase 2: K^T, Q^T (head-pair packed) -----------
                    for hp in range(HP):
                        wkt = wstream.tile([P, EC, P], BF16, tag="wk")
                        nc.sync.dma_start(out=wkt,
                                          in_=wk_d[hp].rearrange("ec e d -> e ec d"))
                        wqt = wstream.tile([P, EC, P], BF16, tag="wq")
                        nc.sync.dma_start(out=wqt,
                                          in_=wq_d[hp].rearrange("ec e d -> e ec d"))
                        for seg in range(4):
                            pk = mm_psum.tile([P, 512], F32, tag="mm")
                            for ec in range(EC):
                                nc.tensor.matmul(pk, wkt[:, ec],
                                                 lnT[:, ec, ts(seg, 512)],
                                                 start=(ec == 0), stop=(ec == EC - 1))
                            nc.vector.tensor_copy(out=KT[:, hp, ts(seg, 512)], in_=pk)
                        for seg in range(2):
                            pq = mm_psum.tile([P, 512], F32, tag="mm")
                            for ec in range(EC):
                                nc.tensor.matmul(pq, wqt[:, ec],
                                                 lnT[:, ec, ts(seg, 512)],
                                                 start=(ec == 0), stop=(ec == EC - 1))
                            nc.vector.tensor_scalar_mul(QT[:, hp, ts(seg, 512)], pq,
                                                        float(HD) ** -0.5)

                    # ---- phase 3: V (all heads) -> DRAM -----------------
                    wvt = wv_pool.tile([P, EC, E], BF16)
                    nc.sync.dma_start(
                        out=wvt, in_=wv_d[:, :, :].rearrange("ec e n -> e ec n"))
                    for ch in range(NCH):
                        for half in range(2):
                            pv = mm_psum.tile([P, 512], F32, tag="mm")
                            for ec in range(EC):
                                nc.tensor.matmul(pv, lnT[:, ec, ts(ch, P)],
                                                 wvt[:, ec, ts(half, 512)],
                                                 start=(ec == 0), stop=(ec == EC - 1))
                            vsb = acts.tile([P, 8, HD], BF16, tag="vsb")
                            nc.vector.tensor_copy(
                                out=vsb, in_=pv.rearrange("p (h d) -> p h d", d=HD))
                            nc.sync.dma_start(
                                out=V_dram[ch, :, 8 * half:8 * (half + 1), 0:HD],
                                in_=vsb)

                # ---- phase 4: attention ---------------------------------
                with tc.tile_pool(name="st_psum", bufs=2, space="PSUM") as st_psum, \
                     tc.tile_pool(name="z_psum", bufs=2, space="PSUM") as z_psum, \
                     tc.tile_pool(name="p_pool", bufs=3) as p_pool, \
                     tc.tile_pool(name="v_pool", bufs=4) as v_pool, \
                     tc.tile_pool(name="l_pool", bufs=3) as l_pool:
                    for hp in range(HP):
                        for j in range(2):
                            zp = [z_psum.tile([P, 512], F32, tag="z", name=f"zp{h}")
                                  for h in range(2)]
                            sched = SCHED[j]
                            for ci, ch in enumerate(sched):
                                st = st_psum.tile([P, 1024], F32, tag="st")
                                nc.tensor.matmul(
                                    st[:, 0:512], KT[0:HD, hp, ts(ch, P)],
                                    QT[0:HD, hp, ts(j, 512)],
                                    start=True, stop=True, tile_position=(0, 0))
                                nc.tensor.matmul(
                                    st[:, 512:1024], KT[HD:P, hp, ts(ch, P)],
                                    QT[HD:P, hp, ts(j, 512)],
                                    start=True, stop=True, tile_position=(HD, 0))
                                slot = (8 if j else 0) + ci
                                pt = p_pool.tile([P, 2, 512], BF16, tag="p")
                                nc.scalar.activation(
                                    out=pt.rearrange("p a b -> p (a b)"), in_=st,
                                    func=AF.Exp, bias=mb_sb[:, slot:slot + 1],
                                    scale=1.0)
                                if ch in DIAG[j]:
                                    nc.gpsimd.affine_select(
                                        out=pt, in_=pt,
                                        compare_op=ALU.is_ge, fill=0.0,
                                        base=512 * j - P * ch,
                                        channel_multiplier=-1,
                                        pattern=[[0, 2], [1, 512]])
                                first, last = ci == 0, ci == len(sched) - 1
                                vt = v_pool.tile([P, 2, HD + 1], BF16, tag="vt")
                                nc.sync.dma_start(
                                    out=vt, in_=V_dram[ch, :, 2 * hp:2 * hp + 2, :])
                                for h in range(2):
                                    nc.tensor.matmul(
                                        zp[h][0:HD + 1], vt[:, h],
                                        pt[:, h], start=first, stop=last)
                            for h in range(2):
                                linv = l_pool.tile([1, 512], F32, tag="linv")
                                nc.vector.reciprocal(out=linv, in_=zp[h][HD:HD + 1, :])
                                lb = l_pool.tile([HD, 512], F32, tag="lb")
                                nc.gpsimd.partition_broadcast(lb, linv)
                                dst_p = (h % 2) * HD
                                nc.vector.tensor_tensor(
                                    out=zT[dst_p:dst_p + HD, hp, ts(j, 512)],
                                    in0=zp[h][0:HD, :], in1=lb, op=ALU.mult)

                # attn pool (lnT/KT/QT/V) freed here
                with tc.tile_pool(name="proj_w", bufs=1) as proj_w, \
                     tc.tile_pool(name="xstream2", bufs=3) as xstream, \
                     tc.tile_pool(name="acts2", bufs=3) as acts, \
                     tc.tile_pool(name="res", bufs=3) as res, \
                     tc.tile_pool(name="tp_psum2", bufs=2, space="PSUM") as tp_psum, \
                     tc.tile_pool(name="mm_psum", bufs=2, space="PSUM") as mm_psum:
                    wpt = proj_w.tile([P, EC, E], BF16)
                    nc.sync.dma_start(out=wpt, in_=wp_d[:, :, :].rearrange("dc d e -> d dc e"))
                    for qc in range(NQ // P):
                        xo = xstream.tile([P, E], F32, tag="x")
                        nc.sync.dma_start(out=xo, in_=xp_d[ts(qc, P), :])
                        o1 = res.tile([P, E], F32, tag="o1")
                        for half in range(2):
                            po = mm_psum.tile([P, 512], F32, tag="mm")
                            for dc in range(EC):
                                nc.tensor.matmul(po, zT[:, dc, ts(qc, P)],
                                                 wpt[:, dc, ts(half, 512)],
                                                 start=(dc == 0), stop=(dc == EC - 1))
                            t1 = res.tile([P, 512], F32, tag="t1")
                            nc.vector.scalar_tensor_tensor(
                                out=t1, in0=po, scalar=0.0,
                                in1=xo[:, ts(half, 512)],
                                op0=ALU.bypass, op1=ALU.add)
                            nc.vector.tensor_tensor(
                                out=o1[:, ts(half, 512)], in0=t1,
                                in1=bp_sb[:, ts(half, 512)], op=ALU.add)
                        nc.sync.dma_start(out=out1_dram[ts(qc, P), :], in_=o1)
                        layernorm_to_T(tc, pools, o1, qc, g2_sb, h2_sb, ln2T,
                                       acts, tp_psum)

            # ---- phase 7: FFN mm1 + relu ---------------------------------
            with tc.tile_pool(name="ffn1", bufs=1) as ffn1, \
                 tc.tile_pool(name="a_pool", bufs=1) as a_pool, \
                 tc.tile_pool(name="mm_psum2", bufs=2, space="PSUM") as mm_psum2:
                a_sb = a_pool.tile([P, FC, NQ], BF16)
                w1t = ffn1.tile([P, EC, FF], BF16)
                nc.sync.dma_start(out=w1t, in_=w1_d[:, :, :].rearrange("ec e f -> e ec f"))
                for fc in range(FC):
                    pa = mm_psum2.tile([P, 1024], F32, tag="pa")
                    for qh in range(2):
                        for ec in range(EC):
                            nc.tensor.matmul(pa[:, ts(qh, 512)],
                                             w1t[:, ec, ts(fc, P)],
                                             ln2T[:, ec, ts(qh, 512)],
                                             start=(ec == 0), stop=(ec == EC - 1))
                    nc.scalar.activation(out=a_sb[:, fc, :], in_=pa, func=AF.Relu,
                                         bias=b1_sb[:, fc:fc + 1], scale=1.0)

                # ---- phase 8: FFN mm2 + residual2 + store ----------------
                with tc.tile_pool(name="ffn2", bufs=1) as ffn2, \
                     tc.tile_pool(name="res2", bufs=3) as res2:
                    w2t = ffn2.tile([P, FC, E], BF16)
                    nc.sync.dma_start(out=w2t, in_=w2_d[:, :, :].rearrange("fc f e -> f fc e"))
                    for qc in range(NQ // P):
                        o1r = res2.tile([P, E], F32, tag="o1r")
                        nc.sync.dma_start(out=o1r, in_=out1_dram[ts(qc, P), :])
                        fin = res2.tile([P, E], F32, tag="fin")
                        for half in range(2):
                            pf = mm_psum2.tile([P, 512], F32, tag="pf")
                            for fc in range(FC):
                                nc.tensor.matmul(pf, a_sb[:, fc, ts(qc, P)],
                                                 w2t[:, fc, ts(half, 512)],
                                                 start=(fc == 0), stop=(fc == FC - 1))
                            t2 = res2.tile([P, 512], F32, tag="t2")
                            nc.vector.scalar_tensor_tensor(
                                out=t2, in0=pf, scalar=0.0,
                                in1=o1r[:, ts(half, 512)],
                                op0=ALU.bypass, op1=ALU.add)
                            nc.vector.tensor_tensor(
                                out=fin[:, ts(half, 512)], in0=t2,
                                in1=b2_sb[:, ts(half, 512)], op=ALU.add)
                        nc.sync.dma_start(out=out_d[ts(qc, P), :], in_=fin)

    nc.compile()
    return nc


def _perms():
    a_own = np.concatenate([np.arange(0, 512), np.arange(1536, 2048)])
    a_rest = np.arange(512, 1536)
    b_own = np.arange(512, 1536)
    b_rest = np.concatenate([np.arange(0, 512), np.arange(1536, 2048)])
    return [np.concatenate([a_own, a_rest]), np.concatenate([b_own, b_rest])], \
           [a_own, b_own]


def _mask_bias():
    mb = [np.zeros(24, np.float32), np.zeros(24, np.float32)]
    mb[0][4:8] = NEG     # role A, tile0, chunks 8-11 (future keys)
    mb[1][20:24] = NEG   # role B, tile1, chunks 12-15 (future keys)
    return mb


def _prep_shared(wq, wk, wv, w_proj, b_proj, w1, b1, w2, b2,
                 ln1_g, ln1_b, ln2_g, ln2_b):
    bf = ml_dtypes.bfloat16

    def pack_pair(w):  # [H, E, HD] -> [H/2, E/P, P, P] bf16
        wpair = w.reshape(H // 2, 2, E, HD)
        cat = np.concatenate([wpair[:, 0], wpair[:, 1]], axis=-1)  # [H/2, E, 128]
        return np.ascontiguousarray(cat.reshape(H // 2, E // P, P, P)).astype(bf)

    shared = {
        "wq2": pack_pair(wq),
        "wk2": pack_pair(wk),
        "wv": np.ascontiguousarray(
            wv.transpose(1, 0, 2).reshape(E // P, P, E)).astype(bf),
        "wp": np.ascontiguousarray(w_proj.reshape(E // P, P, E)).astype(bf),
        "w1": np.ascontiguousarray(w1.reshape(E // P, P, FF)).astype(bf),
        "w2": np.ascontiguousarray(w2.reshape(FF // P, P, E)).astype(bf),
        "b1t": np.ascontiguousarray(b1.reshape(FF // P, P).T).astype(np.float32),
        "bproj": b_proj.astype(np.float32),
        "b2": b2.astype(np.float32),
        "g1t": np.ascontiguousarray(ln1_g.reshape(E // P, P).T).astype(np.float32),
        "h1t": np.ascontiguousarray(ln1_b.reshape(E // P, P).T).astype(np.float32),
        "g2t": np.ascontiguousarray(ln2_g.reshape(E // P, P).T).astype(np.float32),
        "h2t": np.ascontiguousarray(ln2_b.reshape(E // P, P).T).astype(np.float32),
    }
    return shared


def make_in_maps(x, **weights):
    """Build the 8 per-core input dicts (and the gather info)."""
    shared = _prep_shared(**weights)
    perms, owns = _perms()
    mbs = _mask_bias()
    in_maps = []
    for c in range(8):
        b, r = c // 2, c % 2
        m = dict(shared)
        m["xp"] = np.ascontiguousarray(x[b][perms[r]]).astype(np.float32)
        m["mb"] = mbs[r]
        in_maps.append(m)
    return in_maps, owns


def get_nc():
    if "nc" not in _CACHE:
        _CACHE["nc"] = _build_program()
    return _CACHE["nc"]


def kernel(x, wq, wk, wv, w_proj, b_proj, w1, b1, w2, b2,
           ln1_g, ln1_b, ln2_g, ln2_b):
    x = np.asarray(x, dtype=np.float32)
    weights = dict(wq=np.asarray(wq), wk=np.asarray(wk), wv=np.asarray(wv),
                   w_proj=np.asarray(w_proj), b_proj=np.asarray(b_proj),
                   w1=np.asarray(w1), b1=np.asarray(b1), w2=np.asarray(w2),
                   b2=np.asarray(b2), ln1_g=np.asarray(ln1_g),
                   ln1_b=np.asarray(ln1_b), ln2_g=np.asarray(ln2_g),
                   ln2_b=np.asarray(ln2_b))
    nc = get_nc()
    in_maps, owns = make_in_maps(x, **weights)
    res = run_bass_kernel_spmd(nc, in_maps, core_ids=list(range(8)))
    out = np.empty((4, S, E), dtype=np.float32)
    for c in range(8):
        b, r = c // 2, c % 2
        out[b][owns[r]] = res.results[c]["out"]
    return out
